# revision 49
# baseline (speedup 1.0000x reference)
"""Multi-head attention Trainium2 kernel (8 NeuronCores, tensor+data parallel).

Problem: B=2, S=2048, H=1024, NH=16 heads, DH=64, causal additive mask.
  qkv = hs @ w_qkv ; per-head scaled-dot-product attention ; out = ctx @ w_out

Sharding: core c owns batch b=c//4 and 4 heads g=(c%4)*4..+4.  Each core
computes Q^T/K^T for its head slice, V in normal [s,d] layout (directly via
matmul, no transposes), attention in transposed-score layout (softmax along
the PSUM partition axis, sums via a ones-column augmented V), and a partial
out-projection over its 256 head features; the host sums the 4 partials per
batch.

On-device data is bf16 (PE streams bf16 at the same 1 col/cycle as f32r but
weight loads get FWL, and DMA/DVE halve).  PSUM accumulation stays f32.
Inputs are pre-swizzled on the host into the exact SBUF tile layouts so every
DMA is a contiguous 4-12KB-per-partition-row transfer.  Softmax reciprocals
are broadcast across partitions with the GpSimd partition_broadcast custom op
(no DRAM round trip), ctx PSUM tiles are double-buffered, and the QKV /
out-projection matmul bursts are interleaved into the attention k-loops as
fillers so the PE never idles while the Scalar engine runs exp (which also
keeps the PE HAM clock-gate warm at 2.4 GHz).
"""

import sys

sys.path.insert(0, "/opt/trn_rl_repo")

import numpy as np

B, S, H, NH = 2, 2048, 1024, 16
DH = H // NH  # 64
N_CORES = 8
HEADS_PER_CORE = 4  # 2 pairs
SC = S // 512  # 4 q/s chunks of 512
KT = S // 128  # 16 k tiles of 128
F_CORE = HEADS_PER_CORE * DH  # 256 out-proj features per core

_CACHE = {}


def _build(mode, debug_dump=False):
    """Build + schedule the Bass program for `mode` in {"causal", "full"}."""
    import concourse.bass as bass
    import concourse.mybir as mybir
    from concourse import bacc
    from concourse.tile import TileContext

    f32 = mybir.dt.float32
    bf16 = mybir.dt.bfloat16
    EXP = mybir.ActivationFunctionType.Exp

    nc = bacc.Bacc("TRN2", target_bir_lowering=False, debug=False,
                   num_devices=N_CORES)

    # all inputs pre-swizzled to SBUF tile layout on the host
    hT4 = nc.dram_tensor("hT4", [SC, 128, 8 * 512], bf16, kind="ExternalInput").ap()
    # [p, ht, j]; j: [q pair0 (2x64), q pair1, k pair0, k pair1, v (4 heads x 64)]
    wqkv = nc.dram_tensor("wqkv", [128, 8, 6 * 128], bf16, kind="ExternalInput").ap()
    wo = nc.dram_tensor("wo", [128, 2, H], bf16, kind="ExternalInput").ap()
    strip = nc.dram_tensor("strip", [128, 2, 896], bf16, kind="ExternalInput").ap()
    out = nc.dram_tensor("out", [S, H], bf16, kind="ExternalOutput").ap()
    if debug_dump:
        dbg_qkvT = nc.dram_tensor("dbg_qkvT", [SC, 128, 4, 512], bf16, kind="ExternalOutput").ap()
        dbg_v = nc.dram_tensor("dbg_v", [128, HEADS_PER_CORE, KT, 65], bf16, kind="ExternalOutput").ap()
        dbg_r = nc.dram_tensor("dbg_r", [SC, 2, 1, 1024], f32, kind="ExternalOutput").ap()
        dbg_ctxT = nc.dram_tensor("dbg_ctxT", [SC, 128, 2, 512], bf16, kind="ExternalOutput").ap()

    def n_kt(qc):  # k-tiles needed for q chunk qc
        return 4 * qc + 4 if mode == "causal" else KT

    with TileContext(nc) as tc:
        with (
            tc.tile_pool(name="consts", bufs=1) as consts,
            tc.tile_pool(name="persist", bufs=1) as persist,
            tc.tile_pool(name="stream", bufs=2) as stream,
            tc.tile_pool(name="epool", bufs=4) as epool,
            tc.tile_pool(name="norm", bufs=2) as norm,
            tc.tile_pool(name="outp", bufs=4) as outp,
            tc.tile_pool(name="psA", bufs=2, space="PSUM") as psA,
            tc.tile_pool(name="psB", bufs=2, space="PSUM") as psB,
        ):
            # ---- tiles ----------------------------------------------------
            wqkv_sb = consts.tile([128, 8, 6 * 128], bf16, tag="wqkv")
            wo_sb = consts.tile([128, 2, H], bf16, tag="wo")
            strip_sb = None
            if mode == "causal":
                strip_sb = consts.tile([128, 2, 896], bf16, tag="strip", name="strip_sb")
            # q^T/k^T blocks per s-chunk: [p(d, 2 heads), jt, 512]
            # jt: 0,1 = q pairs, 2,3 = k pairs
            qkvT = [persist.tile([128, 4, 512], bf16, name=f"qkvT{sc}", tag=f"qkvT{sc}") for sc in range(SC)]
            # ctx^T blocks per q-chunk: [p(f within pair), pair, 512]
            ctxT = [persist.tile([128, 2, 512], bf16, name=f"ctxT{qc}", tag=f"ctxT{qc}") for qc in range(SC)]
            # v in normal layout + ones column: [p(k within tile), head, kt, 65]
            v_all = persist.tile([128, HEADS_PER_CORE, KT, 65], bf16, tag="v_all")

            def emit_const_loads():
                # spread the gating loads across all three DMA rings so the
                # first qk sweep (wqkv cols 0:128), the v sweep (512:768) and
                # the remaining qk cols arrive concurrently
                nc.scalar.dma_start(out=wqkv_sb[:, :, 0:128], in_=wqkv[:, :, 0:128])
                nc.gpsimd.dma_start(out=wqkv_sb[:, :, 512:768], in_=wqkv[:, :, 512:768])
                nc.scalar.dma_start(out=wqkv_sb[:, :, 128:512], in_=wqkv[:, :, 128:512])
                if strip_sb is not None:
                    nc.gpsimd.dma_start(out=strip_sb, in_=strip)
                nc.gpsimd.dma_start(out=wo_sb, in_=wo)
                # softmax-denominator ones column of the augmented V
                nc.vector.memset(v_all[:, :, :, 64:65], 1.0)

            def load_chunk(sc, split=False, eng=None):
                hT_t = stream.tile([128, 8, 512], bf16, tag="hT", name="hT_t")
                eng = eng or nc.sync
                if split:
                    nc.sync.dma_start(out=hT_t[:, 0:4, :], in_=hT4[sc, :, 0:4 * 512])
                    nc.sync.dma_start(out=hT_t[:, 4:8, :], in_=hT4[sc, :, 4 * 512:8 * 512])
                else:
                    eng.dma_start(out=hT_t, in_=hT4[sc])
                return hT_t

            # ---- filler units: ~1-2us of PE work each ---------------------
            def qk_filler(sc, hT_t, jt):
                def f():
                    ps = psA.tile([128, 1024], f32, tag="psA", name="ps")
                    acc = ps[:, 0:512]
                    for ht in range(8):
                        nc.tensor.matmul(
                            acc,
                            lhsT=wqkv_sb[:, ht, jt * 128:(jt + 1) * 128],
                            rhs=hT_t[:, ht, :],
                            start=(ht == 0),
                            stop=(ht == 7),
                        )
                    nc.vector.tensor_copy(qkvT[sc][:, jt, :], acc)
                return f

            def v_filler(sc, hT_t, i):
                def f():
                    kt = sc * 4 + i
                    pv = psA.tile([128, 1024], f32, tag="psA", name="pv")
                    for ht in range(8):
                        nc.tensor.matmul(
                            pv[:, 0:256],
                            lhsT=hT_t[:, ht, i * 128:(i + 1) * 128],
                            rhs=wqkv_sb[:, ht, 512:768],
                            start=(ht == 0),
                            stop=(ht == 7),
                        )
                    nc.vector.tensor_copy(v_all[:, :, kt, 0:64], pv[:, 0:256])
                return f

            def outproj_filler(blk, i, split_copy=False):
                def f():
                    st = blk * 4 + i
                    po = psA.tile([128, 1024], f32, tag="psA", name="po")
                    # ft-outer: both pair-0 matmuls issue before the pair-1
                    # ones, so they run while pair 1's norm is still finishing
                    for ft in range(2):
                        for ec in range(2):
                            nc.tensor.matmul(
                                po[:, ec * 512:(ec + 1) * 512],
                                lhsT=ctxT[blk][:, ft, i * 128:(i + 1) * 128],
                                rhs=wo_sb[:, ft, ec * 512:(ec + 1) * 512],
                                start=(ft == 0), stop=(ft == 1),
                            )
                    o_sb = outp.tile([128, 1024], bf16, tag="o_sb", name="o_sb")
                    if split_copy:
                        # tail only: ACT is idle there, halve the copy latency
                        nc.vector.tensor_copy(o_sb[:, 0:512], po[:, 0:512])
                        nc.scalar.copy(o_sb[:, 512:1024], po[:, 512:1024])
                    else:
                        nc.vector.tensor_copy(o_sb, po)
                    # alternate store rings so the final block drains 2x faster
                    eng = nc.sync if i % 2 == 0 else nc.gpsimd
                    eng.dma_start(out=out[st * 128:(st + 1) * 128, :], in_=o_sb)
                return f

            def qkv_fillers(sc, hT_t):
                return ([qk_filler(sc, hT_t, jt) for jt in range(4)]
                        + [v_filler(sc, hT_t, i) for i in range(4)])

            def emit_qkv(sc, hT_t):
                # order matched to the staggered arrival of the wqkv pieces
                qk = [qk_filler(sc, hT_t, jt) for jt in range(4)]
                v = [v_filler(sc, hT_t, i) for i in range(4)]
                for f in [qk[0], v[0], v[1], qk[1], v[2], v[3], qk[2], qk[3]]:
                    f()
                if debug_dump:
                    nc.sync.dma_start(out=dbg_qkvT[sc], in_=qkvT[sc][:])

            def emit_attention(qc, fillers):
                nkt = n_kt(qc)
                steps = 2 * nkt
                nf = len(fillers)
                due = [1 + (j * (steps - 1)) // nf for j in range(nf)] if nf else []
                step = 0
                fi = 0
                for pair in range(2):
                    hA, hB = 2 * pair, 2 * pair + 1
                    ctxA = psB.tile([65, 512], f32, tag="ctxA", name="ctxA")
                    ctxB = psB.tile([65, 512], f32, tag="ctxB", name="ctxB")

                    def emit_av(kt, w0, E):
                        nc.tensor.matmul(
                            ctxA[:, w0:512],
                            lhsT=v_all[:, hA, kt, :],
                            rhs=E[:, 0, w0:512],
                            start=(kt == 0), stop=(kt == nkt - 1),
                        )
                        nc.tensor.matmul(
                            ctxB[:, w0:512],
                            lhsT=v_all[:, hB, kt, :],
                            rhs=E[:, 1, w0:512],
                            start=(kt == 0), stop=(kt == nkt - 1),
                        )

                    # software-pipelined: scores(kt) and exp(kt) issue before
                    # AV(kt-1), so the Scalar engine's exps run back-to-back
                    # and AV never waits on a just-issued exp
                    prev = None
                    for kt in range(nkt):
                        # diagonal tiles only need columns j >= 128*t
                        diag = mode == "causal" and kt >= 4 * qc
                        w0 = 128 * (kt - 4 * qc) if diag else 0
                        sp = psA.tile([128, 1024], f32, tag="psA")
                        kblk, ki = qkvT[kt // 4], (kt % 4) * 128
                        # transposed scores, 2 heads row-packed on the PE
                        nc.tensor.matmul(
                            sp[:, w0:512],
                            lhsT=kblk[0:64, 2 + pair, ki:ki + 128],
                            rhs=qkvT[qc][0:64, 0 + pair, w0:512],
                            start=True, stop=True,
                        )
                        nc.tensor.matmul(
                            sp[:, 512 + w0:1024],
                            lhsT=kblk[64:128, 2 + pair, ki:ki + 128],
                            rhs=qkvT[qc][64:128, 0 + pair, w0:512],
                            start=True, stop=True,
                        )
                        E = epool.tile([128, 2, 512], bf16, tag="E")
                        nc.scalar.activation(
                            E[:, :, w0:512],
                            sp[:].rearrange("p (two q) -> p two q", two=2)[:, :, w0:512],
                            EXP)
                        if prev is not None:
                            emit_av(*prev)
                        # PE filler absorbs the remaining exp-period slack
                        while fi < nf and due[fi] <= step:
                            fillers[fi]()
                            fi += 1
                        step += 1
                        if diag:
                            # zero the strictly-masked staircase inside the window
                            nc.vector.tensor_mul(
                                E[:, :, w0:512],
                                E[:, :, w0:512],
                                strip_sb[:, :, 384:896 - w0])
                        prev = (kt, w0, E)
                    emit_av(*prev)
                    # normalization: rows 64 hold the softmax denominators.
                    # the two heads' chains are pipelined, scheduled at high
                    # priority so they don't queue behind filler o_sb copies;
                    # copyB rides the Scalar engine only where ACT has slack
                    act_free = qc < 2 or (qc == SC - 1 and pair == 1)
                    tail = qc == SC - 1 and pair == 1
                    warm = psA.tile([128, 1024], f32, tag="psA", name="warmps") if tail else None
                    with tc.high_priority(offset=40):
                        rA = norm.tile([1, 512], f32, tag="rA", name="rA")
                        rB = norm.tile([1, 512], f32, tag="rB", name="rB")
                        nc.vector.tensor_copy(rA, ctxA[64:65, :])
                        if act_free:
                            nc.scalar.copy(rB, ctxB[64:65, :])
                        else:
                            nc.vector.tensor_copy(rB, ctxB[64:65, :])
                        rr = norm.tile([1, 1024], f32, tag="rr", name="rr")
                        nc.vector.reciprocal_approx_fast(out=rr[0:1, 0:512], in_=rA[:])
                        rbc = norm.tile([128, 1024], f32, tag="rbc", name="rbc")
                        nc.gpsimd.partition_broadcast(rbc[0:64, 0:512], rr[0:1, 0:512], channels=64)
                        if tail:
                            # keep-warm matmuls spaced through the chain by their
                            # deps: without them the >3.4us PE-idle gap here
                            # re-throttles the HAM and the final out-projection
                            # runs at half clock
                            nc.tensor.matmul(warm[:, 0:64], lhsT=rr[0:1, 0:128],
                                             rhs=rr[0:1, 0:64], start=True, stop=True)
                        nc.vector.reciprocal_approx_fast(out=rr[0:1, 512:1024], in_=rB[:])
                        nc.vector.tensor_mul(ctxT[qc][0:64, pair, :], ctxA[0:64, :], rbc[0:64, 0:512])
                        if tail:
                            nc.tensor.matmul(warm[:, 512:576], lhsT=rbc[0:64, 0:128],
                                             rhs=rbc[0:64, 0:64], start=True, stop=True)
                        nc.gpsimd.partition_broadcast(rbc[0:64, 512:1024], rr[0:1, 512:1024], channels=64)
                        nc.vector.tensor_mul(ctxT[qc][64:128, pair, :], ctxB[0:64, :], rbc[0:64, 512:1024])
                    if debug_dump:
                        nc.sync.dma_start(out=dbg_r[qc, pair], in_=rr[:])
                        if pair == 1:
                            nc.sync.dma_start(out=dbg_ctxT[qc], in_=ctxT[qc][:])
                # drain any fillers not yet emitted
                while fi < nf:
                    fillers[fi]()
                    fi += 1

            hT_cur = load_chunk(0, split=True)
            emit_const_loads()
            if mode == "causal":
                # qkv(sc+1) and earlier outproj matmuls are interleaved into
                # attention(sc)'s k-loop as PE fillers; outproj lags one extra
                # phase so the late (ACT-bound) phases get more PE filler work
                emit_qkv(0, hT_cur)
                oblk = 0  # next outproj block to emit
                for sc in range(SC):
                    # chunk 1 rides the scalar ring so it doesn't queue
                    # behind chunk 0's halves on the sync ring
                    hT_next = (load_chunk(sc + 1, eng=nc.scalar if sc == 0 else None)
                               if sc + 1 < SC else None)
                    fillers = []
                    if hT_next is not None:
                        fillers += qkv_fillers(sc + 1, hT_next)
                    # qc2 covers block 0; qc3 (most ACT-bound) covers 1 and 2
                    while oblk < (sc - 1 if sc + 1 < SC else SC - 1):
                        fillers += [outproj_filler(oblk, i) for i in range(4)]
                        oblk += 1
                    emit_attention(sc, fillers)
                    hT_cur = hT_next
                for blk in range(oblk, SC):
                    for i in range(4):
                        outproj_filler(blk, i, split_copy=True)()
                if debug_dump:
                    nc.sync.dma_start(out=dbg_v, in_=v_all[:])
            else:
                emit_qkv(0, hT_cur)
                for sc in range(SC):
                    hT_next = load_chunk(sc + 1) if sc + 1 < SC else None
                    if hT_next is not None:
                        emit_qkv(sc + 1, hT_next)
                    hT_cur = hT_next
                for qc in range(SC):
                    fillers = [outproj_filler(qc - 1, i) for i in range(4)] if qc > 0 else []
                    emit_attention(qc, fillers)
                for i in range(4):
                    outproj_filler(SC - 1, i)()

    nc.compile()
    return nc


def _get_program(mode):
    if mode not in _CACHE:
        _CACHE[mode] = _build(mode)
    return _CACHE[mode]


def _classify_mask(mask):
    """Return "causal", "full", or "generic"."""
    m = mask.reshape(B, S, S)
    tril = np.tril_indices(S)
    if np.all(m == 0.0):
        return "full"
    for b in range(B):
        mb = m[b]
        if not np.all(mb[tril] == 0.0):
            return "generic"
        if not np.all(mb[np.triu_indices(S, k=1)] < -240.0):
            return "generic"
    return "causal"


def _prepare_in_maps(hidden_states, w_qkv, w_out):
    import concourse.mybir as mybir

    bf16 = mybir.dt.np(mybir.dt.bfloat16)

    # strip[i, d, x] = 1.0 iff x >= i + 384 (duplicated along d for head pairs)
    base = (np.arange(896, dtype=np.int32)[None, :] >= (np.arange(128, dtype=np.int32)[:, None] + 384)).astype(np.float32)
    strip = np.ascontiguousarray(np.broadcast_to(base[:, None, :], (128, 2, 896))).astype(bf16)

    # hT4[sc, p, ht*512 + s'] = hs[b, sc*512 + s', ht*128 + p]
    hT4 = [np.ascontiguousarray(
        hidden_states[b].reshape(SC, 512, 8, 128).transpose(0, 3, 2, 1).reshape(SC, 128, 8 * 512)
    ).astype(bf16) for b in range(B)]

    in_maps = []
    for c in range(N_CORES):
        b, g = divmod(c, 4)
        cols = []
        for part in (0, 1):  # q, k column groups of w_qkv (pair-packed)
            for pair in range(2):
                for h in (4 * g + 2 * pair, 4 * g + 2 * pair + 1):
                    cols.append(w_qkv[:, part * H + h * DH: part * H + (h + 1) * DH])
        for h in range(4 * g, 4 * g + 4):  # v columns, head-major
            cols.append(w_qkv[:, 2 * H + h * DH: 2 * H + (h + 1) * DH])
        wqkv_c = np.concatenate(cols, axis=1)
        # fold the 1/sqrt(DH) score scale into the q columns
        wqkv_c = np.ascontiguousarray(wqkv_c)
        wqkv_c[:, 0:256] *= 1.0 / np.sqrt(DH)
        # [p, ht, j] layout
        wqkv_c = wqkv_c.reshape(8, 128, 768).transpose(1, 0, 2)
        wo_c = w_out[g * F_CORE:(g + 1) * F_CORE, :].reshape(2, 128, H).transpose(1, 0, 2)
        in_maps.append({"hT4": hT4[b],
                        "wqkv": np.ascontiguousarray(wqkv_c).astype(bf16),
                        "wo": np.ascontiguousarray(wo_c).astype(bf16),
                        "strip": strip})
    return in_maps


def _run(inputs, trace=False):
    from concourse.bass_utils import run_bass_kernel_spmd

    hidden_states = np.asarray(inputs["hidden_states"], dtype=np.float32)
    mask = np.asarray(inputs["attention_mask"], dtype=np.float32)
    w_qkv = np.asarray(inputs["w_qkv"], dtype=np.float32)
    w_out = np.asarray(inputs["w_out"], dtype=np.float32)

    mode = _classify_mask(mask)
    if mode == "generic":
        return _numpy_reference(hidden_states, mask, w_qkv, w_out), None

    nc = _get_program(mode)
    in_maps = _prepare_in_maps(hidden_states, w_qkv, w_out)
    res = run_bass_kernel_spmd(nc, in_maps, list(range(N_CORES)), trace=trace)
    out = np.zeros((B, S, H), dtype=np.float32)
    for c in range(N_CORES):
        out[c // 4] += res.results[c]["out"].astype(np.float32)
    return out, res


def kernel(**inputs):
    out, _ = _run(inputs, trace=False)
    return out


def kernel_traced(**inputs):
    """Like kernel() but with NTFF profiling; returns (out, BassKernelResults)."""
    return _run(inputs, trace=True)


def _numpy_reference(hidden_states, mask, w_qkv, w_out):
    """Exact fallback for unrecognized masks (slow, chunked numpy)."""
    out = np.zeros((B, S, H), dtype=np.float32)
    m = mask.reshape(B, 1, S, S)
    for b in range(B):
        qkv = hidden_states[b] @ w_qkv  # [S, 3H]
        q = qkv[:, 0:H].reshape(S, NH, DH)
        k = qkv[:, H:2 * H].reshape(S, NH, DH)
        v = qkv[:, 2 * H:].reshape(S, NH, DH)
        ctx = np.zeros((S, NH, DH), dtype=np.float32)
        for h in range(NH):
            s = (q[:, h] @ k[:, h].T) / np.sqrt(DH) + m[b, 0]
            s = s - s.max(axis=-1, keepdims=True)
            e = np.exp(s)
            p = e / e.sum(axis=-1, keepdims=True)
            ctx[:, h] = p @ v[:, h]
        out[b] = ctx.reshape(S, H) @ w_out
    return out
